# revision 18
# baseline (speedup 1.0000x reference)
"""Trainium2 Bass kernel for nn_DCT2D_Layer: 8x8 block 2D-DCT + zigzag feature map.

Input : img  [16, 3, 512, 512] f32
Output: feat [16, 192, 64, 64]  f32  where feat[b, c*64+k, ib, jb] is the
        k-th zigzag DCT coefficient of the 8x8 block (ib, jb) of channel c.

Strategy (per core; 8 cores, pure data parallel over the 48 (b,c) images):
  - For each 128x128 image tile X (rows (ib,h), cols (jb,w)):
      mm1: out1 = X.T @ R1   -> out1[(jb,w), n1(u,ib)]  (X is the stationary operand)
      mm2: out2 = out1.T @ R2 -> out2[n1(u,ib), (v,jb)] (out1 is stationary)
    where R1[8*ib+h, n1(u,ib')] = C[u,h] * (ib==ib') and
          R2[8*jb+w, 16*v+jb'] = C[v,w] * (jb==jb') are 128x128 block-diagonal
    arrangements of the 8x8 DCT-II basis C.  n1(u, ib) = 8*ib + u spreads each
    frequency u over stride-8 partitions (more SDMA engines per store DMA).
  - Precision: hi/lo bf16 split (x = xh + xl, C = ch + cl), computing
    xh@ch + xl@ch + xh@cl accumulated in fp32 PSUM; the out1 intermediate is
    re-split on DVE.  End-to-end ~1e-5 relative error at bf16 matmul speed.
  - out2 is copied PSUM->SBUF into a v-major staging layout
    [128 part=n1, v, tI, tJ, jb]; for each frequency pair (u,v) one affine DMA
    writes the full 64x64 map of channel zig(u,v) (contiguous per channel).
    Store DMAs round-robin over the three descriptor generators
    (sync-HWDGE, scalar-HWDGE, gpsimd-SWDGE).
"""

import numpy as np
import ml_dtypes

import concourse.bacc as bacc
import concourse.bass as bass
import concourse.mybir as mybir
from concourse.tile import TileContext
from concourse.bass_utils import run_bass_kernel_spmd

N_CORES = 8
IMGS_TOTAL = 48          # 16 batches x 3 channels
IMGS_PER_CORE = IMGS_TOTAL // N_CORES   # 6
H = W = 512
B = 8                    # DCT block size
NT = 4                   # 128x128 tiles per image side
STRIDE8 = True           # n1(u, ib) = 8*ib + u partition arrangement


def _zigzag(n):
    idx = np.zeros(n * n, dtype=np.int64)
    i = j = 0
    for k in range(n * n):
        idx[k] = i * n + j
        if (i + j) % 2 == 0:
            if j == n - 1:
                i += 1
            elif i == 0:
                j += 1
            else:
                i -= 1
                j += 1
        else:
            if i == n - 1:
                j += 1
            elif j == 0:
                i += 1
            else:
                i += 1
                j -= 1
    return idx


def _dct_basis(N):
    k = np.arange(N)[:, None].astype(np.float64)
    nn = np.arange(N)[None, :].astype(np.float64)
    return (2.0 * np.cos(np.pi * (2.0 * nn + 1.0) * k / (2.0 * N))).astype(np.float32)


def _constants():
    C = _dct_basis(B)
    R1 = np.zeros((128, 128), np.float32)
    R2 = np.zeros((128, 128), np.float32)
    for blk in range(16):
        for x in range(8):
            for f in range(8):
                if STRIDE8:
                    R1[8 * blk + x, 8 * blk + f] = C[f, x]
                else:
                    R1[8 * blk + x, 16 * f + blk] = C[f, x]
                R2[8 * blk + x, 16 * f + blk] = C[f, x]
    zz = _zigzag(B)
    ch_of_flat = np.empty(64, np.int64)
    ch_of_flat[zz] = np.arange(64)
    return R1, R2, ch_of_flat


R1_NP, R2_NP, CH_OF_FLAT = _constants()


def _split_hi_lo(a):
    hi = a.astype(ml_dtypes.bfloat16)
    lo = (a - hi.astype(np.float32)).astype(ml_dtypes.bfloat16)
    return hi, lo


R1H_NP, R1L_NP = _split_hi_lo(R1_NP)
R2H_NP, R2L_NP = _split_hi_lo(R2_NP)
RC_NP = np.concatenate([R1H_NP, R1L_NP, R2H_NP, R2L_NP], axis=1)


def build_kernel(n_imgs=IMGS_PER_CORE):
    f32 = mybir.dt.float32
    bf16 = mybir.dt.bfloat16
    nc = bacc.Bacc("TRN2", target_bir_lowering=False, debug=False,
                   num_devices=N_CORES)

    imgh = nc.dram_tensor("imgh", [n_imgs, H, W], bf16, kind="ExternalInput")
    imgl = nc.dram_tensor("imgl", [n_imgs, H, W], bf16, kind="ExternalInput")
    # rc = [r1h | r1l | r2h | r2l] packed as one [128, 512] constant
    rc = nc.dram_tensor("rc", [128, 512], bf16, kind="ExternalInput")
    out = nc.dram_tensor("out", [n_imgs, 64, 64, 64], f32, kind="ExternalOutput")

    with TileContext(nc) as tc:
        with (
            tc.tile_pool(name="consts", bufs=1) as cpool,
            tc.tile_pool(name="ims", bufs=1) as impool,
            tc.tile_pool(name="o1", bufs=3) as o1pool,
            tc.tile_pool(name="outsb", bufs=3) as opool,
            tc.tile_pool(name="ps1", bufs=3, space="PSUM") as ps1,
            tc.tile_pool(name="ps2", bufs=3, space="PSUM") as ps2,
        ):
            rct = cpool.tile([128, 512], bf16)
            nc.sync.dma_start(out=rct, in_=rc.ap())
            r1ht = rct[:, 0:128]
            r1lt = rct[:, 128:256]
            r2ht = rct[:, 256:384]
            r2lt = rct[:, 384:512]

            # load the core's whole input shard up front (48 KB/partition)
            rings = (nc.sync, nc.scalar, nc.gpsimd)
            imhs, imls = [], []
            for i in range(n_imgs):
                imh = impool.tile([128, NT, W], bf16, tag=f"imh{i}")
                rings[i % 3].dma_start(
                    out=imh, in_=imgh.ap()[i].rearrange("(s p) w -> p s w", p=128)
                )
                iml = impool.tile([128, NT, W], bf16, tag=f"iml{i}")
                rings[(i + 1) % 3].dma_start(
                    out=iml, in_=imgl.ap()[i].rearrange("(s p) w -> p s w", p=128)
                )
                imhs.append(imh)
                imls.append(iml)

            store_n = 0
            for p in range(NT):          # tI-major phases
                # staging: [part=n1(u,ib), v, img, tJ, jb]
                outsb = opool.tile([128, 8, n_imgs, NT, 16], f32)
                for i in range(n_imgs):
                    # quad-wide PSUM tiles: all 4 tJ tiles of (p, i) share one
                    # [128, 512] tile so the DVE copies amortize fixed costs
                    p1q = ps1.tile([128, 512], f32)
                    for tJ in range(NT):
                        xh = imhs[i][:, p, 128 * tJ:128 * (tJ + 1)]
                        xl = imls[i][:, p, 128 * tJ:128 * (tJ + 1)]
                        sl = slice(128 * tJ, 128 * (tJ + 1))
                        # out1 = xh@r1h + xh@r1l + xl@r1h (fp32 accumulate)
                        nc.tensor.matmul(p1q[:, sl], xh, r1ht[:],
                                         start=True, stop=False)
                        nc.tensor.matmul(p1q[:, sl], xh, r1lt[:],
                                         start=False, stop=False)
                        nc.tensor.matmul(p1q[:, sl], xl, r1ht[:],
                                         start=False, stop=True)

                    o1h = o1pool.tile([128, 512], bf16, tag="o1h")
                    o1l = o1pool.tile([128, 512], bf16, tag="o1l")
                    nc.vector.tensor_copy(out=o1h[:], in_=p1q[:])
                    nc.vector.tensor_sub(out=o1l[:], in0=p1q[:], in1=o1h[:])

                    p2q = ps2.tile([128, 512], f32)
                    for tJ in range(NT):
                        sl = slice(128 * tJ, 128 * (tJ + 1))
                        nc.tensor.matmul(p2q[:, sl], o1h[:, sl], r2ht[:],
                                         start=True, stop=False)
                        nc.tensor.matmul(p2q[:, sl], o1h[:, sl], r2lt[:],
                                         start=False, stop=False)
                        nc.tensor.matmul(p2q[:, sl], o1l[:, sl], r2ht[:],
                                         start=False, stop=True)

                    # one wide copy per (p, i): [128, (tJ v j)] -> [128, v, tJ, j]
                    dst2 = outsb[:, :, i, :, :]
                    src2 = p2q[:].rearrange("p (t v j) -> p v t j", t=NT, v=8)
                    nc.scalar.copy(out=dst2, in_=src2)
                # one DMA per frequency pair (u, v) covering ALL images' phase-p
                # row block: src [ib(16 part), img(6), (tJ jb)=64 contiguous]
                # dst out[:, zig(u,v), 16p:16p+16, :] viewed as [ib, img, w]
                for u in range(8):
                    for v in range(8):
                        k = int(CH_OF_FLAT[u * 8 + v])
                        if STRIDE8:
                            src = outsb[u::8, v, :, :, :]
                        else:
                            src = outsb[16 * u:16 * (u + 1), v, :, :, :]
                        dst = out.ap()[:, k, 16 * p:16 * (p + 1), :].rearrange(
                            "i ib w -> ib i w"
                        )
                        eng = (nc.sync, nc.gpsimd, nc.scalar)[store_n % 3]
                        eng.dma_start(out=dst, in_=src)
                        store_n += 1

    nc.compile()
    return nc


_NC_CACHE = {}


def _get_nc(n_imgs):
    if n_imgs not in _NC_CACHE:
        _NC_CACHE[n_imgs] = build_kernel(n_imgs)
    return _NC_CACHE[n_imgs]


def _in_maps(flat):
    hi = flat.astype(ml_dtypes.bfloat16)
    lo = (flat - hi.astype(np.float32)).astype(ml_dtypes.bfloat16)
    maps = []
    for cid in range(N_CORES):
        sl = slice(IMGS_PER_CORE * cid, IMGS_PER_CORE * (cid + 1))
        maps.append({
            "imgh": np.ascontiguousarray(hi[sl]),
            "imgl": np.ascontiguousarray(lo[sl]),
            "rc": RC_NP,
        })
    return maps


def run(img, trace=False):
    """img: [16,3,512,512] f32 -> (feat [16,192,64,64] f32, BassKernelResults)."""
    img = np.ascontiguousarray(np.asarray(img), dtype=np.float32)
    bs, c, h, w = img.shape
    flat = img.reshape(bs * c, h, w)
    nc = _get_nc(IMGS_PER_CORE)
    res = run_bass_kernel_spmd(nc, _in_maps(flat),
                               core_ids=list(range(N_CORES)), trace=trace)
    shards = [res.results[cid]["out"] for cid in range(N_CORES)]
    feat = np.concatenate(shards, axis=0).reshape(bs, c * 64, 64, 64)
    return feat, res


def kernel(img):
    feat, _ = run(img, trace=False)
    return feat


# revision 21
# speedup vs baseline: 1.1398x; 1.1398x over previous
"""Trainium2 Bass kernel for nn_DCT2D_Layer: 8x8 block 2D-DCT + zigzag feature map.

Input : img  [16, 3, 512, 512] f32
Output: feat [16, 192, 64, 64]  f32  where feat[b, c*64+k, ib, jb] is the
        k-th zigzag DCT coefficient of the 8x8 block (ib, jb) of channel c.

Strategy (per core; 8 cores, pure data parallel over the 48 (b,c) images):
  - For each 128x128 image tile X (rows (ib,h), cols (jb,w)):
      mm1: out1 = X.T @ R1   -> out1[(jb,w), n1(u,ib)]  (X is the stationary operand)
      mm2: out2 = out1.T @ R2 -> out2[n1(u,ib), (v,jb)] (out1 is stationary)
    where R1[8*ib+h, n1(u,ib')] = C[u,h] * (ib==ib') and
          R2[8*jb+w, 16*v+jb'] = C[v,w] * (jb==jb') are 128x128 block-diagonal
    arrangements of the 8x8 DCT-II basis C.  n1(u, ib) = 8*ib + u spreads each
    frequency u over stride-8 partitions (more SDMA engines per store DMA).
  - Precision: hi/lo bf16 split (x = xh + xl, C = ch + cl), computing
    xh@ch + xl@ch + xh@cl accumulated in fp32 PSUM; the out1 intermediate is
    re-split on DVE.  End-to-end ~1e-5 relative error at bf16 matmul speed.
  - out2 is copied PSUM->SBUF into a v-major staging layout
    [128 part=n1, v, tI, tJ, jb]; for each frequency pair (u,v) one affine DMA
    writes the full 64x64 map of channel zig(u,v) (contiguous per channel).
    Store DMAs round-robin over the three descriptor generators
    (sync-HWDGE, scalar-HWDGE, gpsimd-SWDGE).
"""

import numpy as np
import ml_dtypes

import concourse.bacc as bacc
import concourse.bass as bass
import concourse.mybir as mybir
from concourse.tile import TileContext
from concourse.bass_utils import run_bass_kernel_spmd

N_CORES = 8
IMGS_TOTAL = 48          # 16 batches x 3 channels
IMGS_PER_CORE = IMGS_TOTAL // N_CORES   # 6
H = W = 512
B = 8                    # DCT block size
NT = 4                   # 128x128 tiles per image side
STRIDE8 = True           # n1(u, ib) = 8*ib + u partition arrangement


def _zigzag(n):
    idx = np.zeros(n * n, dtype=np.int64)
    i = j = 0
    for k in range(n * n):
        idx[k] = i * n + j
        if (i + j) % 2 == 0:
            if j == n - 1:
                i += 1
            elif i == 0:
                j += 1
            else:
                i -= 1
                j += 1
        else:
            if i == n - 1:
                j += 1
            elif j == 0:
                i += 1
            else:
                i += 1
                j -= 1
    return idx


def _dct_basis(N):
    k = np.arange(N)[:, None].astype(np.float64)
    nn = np.arange(N)[None, :].astype(np.float64)
    return (2.0 * np.cos(np.pi * (2.0 * nn + 1.0) * k / (2.0 * N))).astype(np.float32)


def _constants():
    C = _dct_basis(B)
    R1 = np.zeros((128, 128), np.float32)
    R2 = np.zeros((128, 128), np.float32)
    for blk in range(16):
        for x in range(8):
            for f in range(8):
                if STRIDE8:
                    R1[8 * blk + x, 8 * blk + f] = C[f, x]
                else:
                    R1[8 * blk + x, 16 * f + blk] = C[f, x]
                R2[8 * blk + x, 16 * f + blk] = C[f, x]
    zz = _zigzag(B)
    ch_of_flat = np.empty(64, np.int64)
    ch_of_flat[zz] = np.arange(64)
    return R1, R2, ch_of_flat


R1_NP, R2_NP, CH_OF_FLAT = _constants()


def _split_hi_lo(a):
    hi = a.astype(ml_dtypes.bfloat16)
    lo = (a - hi.astype(np.float32)).astype(ml_dtypes.bfloat16)
    return hi, lo


R1H_NP, R1L_NP = _split_hi_lo(R1_NP)
R2H_NP, R2L_NP = _split_hi_lo(R2_NP)
RC_NP = np.concatenate([R1H_NP, R1L_NP, R2H_NP, R2L_NP], axis=1)


def build_kernel(n_imgs=IMGS_PER_CORE):
    f32 = mybir.dt.float32
    bf16 = mybir.dt.bfloat16
    nc = bacc.Bacc("TRN2", target_bir_lowering=False, debug=False,
                   num_devices=N_CORES)

    imgh = nc.dram_tensor("imgh", [n_imgs, H, W], bf16, kind="ExternalInput")
    imgl = nc.dram_tensor("imgl", [n_imgs, H, W], bf16, kind="ExternalInput")
    # rc = [r1h | r1l | r2h | r2l] packed as one [128, 512] constant
    rc = nc.dram_tensor("rc", [128, 512], bf16, kind="ExternalInput")
    out = nc.dram_tensor("out", [n_imgs, 64, 64, 64], f32, kind="ExternalOutput")

    with TileContext(nc) as tc:
        with (
            tc.tile_pool(name="consts", bufs=1) as cpool,
            tc.tile_pool(name="ims", bufs=1) as impool,
            tc.tile_pool(name="o1", bufs=4) as o1pool,
            tc.tile_pool(name="outsb", bufs=3) as opool,
            tc.tile_pool(name="ps1", bufs=4, space="PSUM") as ps1,
            tc.tile_pool(name="ps2", bufs=4, space="PSUM") as ps2,
        ):
            rct = cpool.tile([128, 512], bf16)
            nc.sync.dma_start(out=rct, in_=rc.ap())
            r1ht = rct[:, 0:128]
            r1lt = rct[:, 128:256]
            r2ht = rct[:, 256:384]
            r2lt = rct[:, 384:512]

            # HAM warmup: ~6 junk matmuls (~3.8 us cold) flip the PE clock to
            # 2.4 GHz just before the real phase-0 matmuls start (they only
            # depend on the tiny rc const load, so they run during the input
            # DMAs).  Shares a ps1 slot (tag) so no extra PSUM bank is used.
            warm = ps1.tile([128, 512], f32, tag="p1q")
            for wi in range(6):
                nc.tensor.matmul(warm[:], rct[:, 0:128], rct[:],
                                 start=(wi == 0), stop=(wi == 5))

            # load the core's whole input shard up front (48 KB/partition)
            rings = (nc.sync, nc.scalar, nc.gpsimd)
            imhs, imls = [], []
            for i in range(n_imgs):
                imh = impool.tile([128, NT, W], bf16, tag=f"imh{i}")
                rings[i % 3].dma_start(
                    out=imh, in_=imgh.ap()[i].rearrange("(s p) w -> p s w", p=128)
                )
                iml = impool.tile([128, NT, W], bf16, tag=f"iml{i}")
                rings[(i + 1) % 3].dma_start(
                    out=iml, in_=imgl.ap()[i].rearrange("(s p) w -> p s w", p=128)
                )
                imhs.append(imh)
                imls.append(iml)

            store_n = 0
            for p in range(NT):          # tI-major phases
                # staging: [part=n1(u,ib), v, img, tJ, jb]
                outsb = opool.tile([128, 8, n_imgs, NT, 16], f32)
                for i in range(n_imgs):
                    # quad-wide PSUM tiles: all 4 tJ tiles of (p, i) share one
                    # [128, 512] tile so the DVE copies amortize fixed costs
                    p1q = ps1.tile([128, 512], f32)
                    for tJ in range(NT):
                        xh = imhs[i][:, p, 128 * tJ:128 * (tJ + 1)]
                        xl = imls[i][:, p, 128 * tJ:128 * (tJ + 1)]
                        sl = slice(128 * tJ, 128 * (tJ + 1))
                        # out1 = xh@r1h + xh@r1l + xl@r1h (fp32 accumulate)
                        nc.tensor.matmul(p1q[:, sl], xh, r1ht[:],
                                         start=True, stop=False)
                        nc.tensor.matmul(p1q[:, sl], xh, r1lt[:],
                                         start=False, stop=False)
                        nc.tensor.matmul(p1q[:, sl], xl, r1ht[:],
                                         start=False, stop=True)

                    o1h = o1pool.tile([128, 512], bf16, tag="o1h")
                    o1l = o1pool.tile([128, 512], bf16, tag="o1l")
                    nc.vector.tensor_copy(out=o1h[:], in_=p1q[:])
                    nc.vector.tensor_sub(out=o1l[:], in0=p1q[:], in1=o1h[:])

                    p2q = ps2.tile([128, 512], f32)
                    for tJ in range(NT):
                        sl = slice(128 * tJ, 128 * (tJ + 1))
                        nc.tensor.matmul(p2q[:, sl], o1h[:, sl], r2ht[:],
                                         start=True, stop=False)
                        nc.tensor.matmul(p2q[:, sl], o1h[:, sl], r2lt[:],
                                         start=False, stop=False)
                        nc.tensor.matmul(p2q[:, sl], o1l[:, sl], r2ht[:],
                                         start=False, stop=True)

                    # one wide copy per (p, i): [128, (tJ v j)] -> [128, v, tJ, j]
                    dst2 = outsb[:, :, i, :, :]
                    src2 = p2q[:].rearrange("p (t v j) -> p v t j", t=NT, v=8)
                    nc.vector.tensor_copy(out=dst2, in_=src2)
                # one DMA per frequency pair (u, v) covering ALL images' phase-p
                # row block: src [ib(16 part), img(6), (tJ jb)=64 contiguous]
                # dst out[:, zig(u,v), 16p:16p+16, :] viewed as [ib, img, w]
                for u in range(8):
                    for v in range(8):
                        k = int(CH_OF_FLAT[u * 8 + v])
                        if STRIDE8:
                            src = outsb[u::8, v, :, :, :]
                        else:
                            src = outsb[16 * u:16 * (u + 1), v, :, :, :]
                        dst = out.ap()[:, k, 16 * p:16 * (p + 1), :].rearrange(
                            "i ib w -> ib i w"
                        )
                        eng = (nc.sync, nc.gpsimd, nc.scalar)[store_n % 3]
                        eng.dma_start(out=dst, in_=src)
                        store_n += 1

    nc.compile()
    return nc


_NC_CACHE = {}


def _get_nc(n_imgs):
    if n_imgs not in _NC_CACHE:
        _NC_CACHE[n_imgs] = build_kernel(n_imgs)
    return _NC_CACHE[n_imgs]


def _in_maps(flat):
    hi = flat.astype(ml_dtypes.bfloat16)
    lo = (flat - hi.astype(np.float32)).astype(ml_dtypes.bfloat16)
    maps = []
    for cid in range(N_CORES):
        sl = slice(IMGS_PER_CORE * cid, IMGS_PER_CORE * (cid + 1))
        maps.append({
            "imgh": np.ascontiguousarray(hi[sl]),
            "imgl": np.ascontiguousarray(lo[sl]),
            "rc": RC_NP,
        })
    return maps


def run(img, trace=False):
    """img: [16,3,512,512] f32 -> (feat [16,192,64,64] f32, BassKernelResults)."""
    img = np.ascontiguousarray(np.asarray(img), dtype=np.float32)
    bs, c, h, w = img.shape
    flat = img.reshape(bs * c, h, w)
    nc = _get_nc(IMGS_PER_CORE)
    res = run_bass_kernel_spmd(nc, _in_maps(flat),
                               core_ids=list(range(N_CORES)), trace=trace)
    shards = [res.results[cid]["out"] for cid in range(N_CORES)]
    feat = np.concatenate(shards, axis=0).reshape(bs, c * 64, 64, 64)
    return feat, res


def kernel(img):
    feat, _ = run(img, trace=False)
    return feat
